# revision 49
# baseline (speedup 1.0000x reference)
"""Trainium2 Bass kernel for nn_AttentionModule (dense_transformer).

Computes, per batch b:
  q = LN(gelu([student_o;student_d] @ Wq + bq)) * gq + betaq      [B, L, D]
  k = LN(gelu([teacher_o;teacher_d] @ Wk + bk)) * gk + betak      [B, M, D]
  scores = q @ k^T / sqrt(D) + depth_bias                          [B, L, M]
  alpha  = softmax(scores, axis=-1)
  z      = alpha @ basis                                           [B, L, D]
Returns (z, alpha).

Strategy: data-parallel over batch across 8 NeuronCores (64 batches/core).
All matmuls in bf16 (fp32 PSUM accumulation); activations are cast
fp32->bf16 during the SWDGE DMA load and transposed on the PE
(features-on-partitions) since the TensorE contracts along partitions.
1/sqrt(D) is folded into gq/betaq host-side; the projection bias is folded
into the matmul as an extra K=1 rank-1 update (ones x bias).
"""

import sys
import numpy as np

for _p in ("/opt/trn_rl_repo", "/opt/pypackages"):
    if _p not in sys.path:
        sys.path.append(_p)

# ---- problem constants (hardcoded per contract) ----
B, L, M = 512, 32, 48
SH, TH, D = 1024, 4096, 512
LAMBDA_POS = 1.0
EPS = 1e-5
NCORES = 8
BPC = B // NCORES            # 64 batches per core
RQ = BPC * L                 # 2048 q rows per core
RK = BPC * M                 # 3072 k rows per core
FQ = 2 * SH                  # 2048
FK = 2 * TH                  # 8192
NQB = RQ // 128              # 16 q row-blocks
NKB = RK // 128              # 24 k row-blocks
NQF = FQ // 128              # 16 q feature tiles
NKF = FK // 128              # 64 k feature tiles
ND = D // 128                # 4 d tiles

_NC = None


def _build_nc():
    import concourse.tile as tile
    from concourse import bacc, mybir, masks

    fp32 = mybir.dt.float32
    bf16 = mybir.dt.bfloat16
    AF = mybir.ActivationFunctionType
    ALU = mybir.AluOpType
    AX = mybir.AxisListType

    nc = bacc.Bacc("TRN2", target_bir_lowering=False, debug=False,
                   enable_asserts=False, num_devices=NCORES)

    # ---- DRAM I/O ----
    # activations arrive pre-swizzled from the host as [block, f, ftile, row]
    # so a single contiguous DMA per block yields feature-on-partitions tiles
    # (no on-chip transpose needed).
    so_d = nc.dram_tensor("student_oT", [NQB, 128, SH // 128, 128], bf16,
                          kind="ExternalInput").ap()
    sd_d = nc.dram_tensor("student_dT", [NQB, 128, SH // 128, 128], bf16,
                          kind="ExternalInput").ap()
    to_d = nc.dram_tensor("teacher_oT", [NKB, 128, TH // 128, 128], bf16,
                          kind="ExternalInput").ap()
    td_d = nc.dram_tensor("teacher_dT", [NKB, 128, TH // 128, 128], bf16,
                          kind="ExternalInput").ap()
    wq_d = nc.dram_tensor("Wq", [FQ, D], bf16, kind="ExternalInput").ap()
    wk_d = nc.dram_tensor("Wk", [FK, D], bf16, kind="ExternalInput").ap()
    bq_d = nc.dram_tensor("bq", [D], fp32, kind="ExternalInput").ap()
    bk_d = nc.dram_tensor("bk", [D], fp32, kind="ExternalInput").ap()
    gq_d = nc.dram_tensor("gq_s", [D], fp32, kind="ExternalInput").ap()
    betaq_d = nc.dram_tensor("betaq_s", [D], fp32, kind="ExternalInput").ap()
    gk_d = nc.dram_tensor("gk", [D], fp32, kind="ExternalInput").ap()
    betak_d = nc.dram_tensor("betak", [D], fp32, kind="ExternalInput").ap()
    basis_d = nc.dram_tensor("basis", [M, D], bf16, kind="ExternalInput").ap()
    dbias_d = nc.dram_tensor("depth_bias", [128, M], fp32, kind="ExternalInput").ap()
    z_d = nc.dram_tensor("z", [BPC, L, D], fp32, kind="ExternalOutput").ap()
    al_d = nc.dram_tensor("alpha", [BPC, L, M], fp32, kind="ExternalOutput").ap()

    z_f = z_d.flatten_outer_dims()     # [2048, 512]
    al_f = al_d.flatten_outer_dims()   # [2048, 48]

    import contextlib
    with tile.TileContext(nc) as tc, contextlib.ExitStack() as ctx:
        const = ctx.enter_context(tc.tile_pool(name="const", bufs=1))
        stage_p = ctx.enter_context(tc.tile_pool(name="stage", bufs=1))
        qkt_p = ctx.enter_context(tc.tile_pool(name="qkt", bufs=1))

        # ---- constants ----
        ident = const.tile([128, 128], bf16)
        masks.make_identity(nc, ident[:])
        eps_sb = const.tile([128, 1], fp32)
        nc.gpsimd.memset(eps_sb[:], float(EPS))

        # weight tiles are allocated per SOURCE (finer dependency grains);
        # their DMAs are issued inside phase 1 (Wq right after the first
        # activation loads, Wk chunks streamed during the Q phase) to avoid
        # a PE startup stall.
        wqo_sb = const.tile([128, SH // 128, D], bf16)   # [128, 8, 512]
        wqd_sb = const.tile([128, SH // 128, D], bf16)
        wko_sb = const.tile([128, TH // 128, D], bf16)   # [128, 32, 512]
        wkd_sb = const.tile([128, TH // 128, D], bf16)

        wq_r = wq_d.rearrange("(t p) d -> p t d", p=128)   # [128, 16, 512]
        wk_r = wk_d.rearrange("(t p) d -> p t d", p=128)   # [128, 64, 512]

        def load_wq_chunk(c):   # c in 0..1 -> one source each
            dst = (wqo_sb, wqd_sb)[c]
            nc.gpsimd.dma_start(out=dst[:], in_=wq_r[:, c * 8:(c + 1) * 8, :])

        def load_wk_chunk(c):   # c in 0..7; 0-3 -> wko, 4-7 -> wkd
            dst = wko_sb if c < 4 else wkd_sb
            cc = c % 4
            nc.gpsimd.dma_start(out=dst[:, cc * 8:(cc + 1) * 8, :],
                                in_=wk_r[:, c * 8:(c + 1) * 8, :])

        import concourse.bass as bass

        def bcast128(src_ap, n):
            return bass.AP(tensor=src_ap.tensor, offset=src_ap.offset,
                           ap=[[0, 128]] + src_ap.ap)

        bq_bc = const.tile([128, D], fp32)
        nc.gpsimd.dma_start(out=bq_bc[:], in_=bcast128(bq_d, D))
        bk_bc = const.tile([128, D], fp32)
        nc.gpsimd.dma_start(out=bk_bc[:], in_=bcast128(bk_d, D))
        # gamma/beta in transposed (d-on-partitions) layout: [128, ND]
        gqT = const.tile([128, ND], fp32)
        nc.gpsimd.dma_start(out=gqT[:], in_=gq_d.rearrange("(t p) -> p t", p=128))
        betaqT = const.tile([128, ND], fp32)
        nc.gpsimd.dma_start(out=betaqT[:],
                            in_=betaq_d.rearrange("(t p) -> p t", p=128))
        gkT = const.tile([128, ND], fp32)
        nc.gpsimd.dma_start(out=gkT[:], in_=gk_d.rearrange("(t p) -> p t", p=128))
        betakT = const.tile([128, ND], fp32)
        nc.gpsimd.dma_start(out=betakT[:],
                            in_=betak_d.rearrange("(t p) -> p t", p=128))
        basis_sb = const.tile([M, D], bf16)
        nc.gpsimd.dma_start(out=basis_sb[:], in_=basis_d)
        dbias_sb = const.tile([128, M], fp32)
        nc.gpsimd.dma_start(out=dbias_sb[:], in_=dbias_d)

        # ---- persistent stage / output tensors (Q and K share one buffer:
        # Q uses rows [0,16), fully consumed before K writes rows [0,24)) ----
        stage = stage_p.tile([128, NKB, D], bf16)      # 24 KB/part
        mv = stage_p.tile([128, NKB, 2], fp32)
        qT = qkt_p.tile([128, ND, RQ], bf16)           # q^T (d on part), 16 KB/part
        kT = qkt_p.tile([128, ND, RK], bf16)           # k^T, 24 KB/part

        drain_eng = [0]

        def drain(out_ap, in_ap):
            # alternate PSUM->SBUF drains between ACT and DVE
            if drain_eng[0] % 2 == 0:
                nc.scalar.activation(out_ap, in_ap, AF.Copy)
            else:
                nc.vector.tensor_copy(out_ap, in_ap)
            drain_eng[0] += 1

        # ================= phase 1: projections + gelu + stats ============
        with tc.tile_pool(name="nat", bufs=5) as nat_p, \
             tc.tile_pool(name="ln", bufs=3) as ln_p, \
             tc.tile_pool(name="prps", bufs=3, space="PSUM") as pr_ps, \
             tc.tile_pool(name="ltps", bufs=2, space="PSUM") as lt_ps:

            def load_ft(src, r, nhf, nchunks=1):
                # contiguous cast-DMA(s): [128 f, nhf*128 rows] bf16 tile,
                # feature already on partitions (host pre-swizzled)
                ft = nat_p.tile([128, nhf * 128], bf16, tag="nat")
                step = nhf // nchunks
                for c in range(nchunks):
                    nc.gpsimd.dma_start(
                        out=ft[:, c * step * 128:(c + 1) * step * 128],
                        in_=src[r, :, c * step:(c + 1) * step, :].rearrange(
                            "p t r -> p (t r)"))
                return ft

            def proj_block(r, nhf, srcs, w_sbs, b_bc, pre=None):
                # per source tensor: accumulate the projection over feature
                # tiles straight out of the DMA-ed featT tile; then bias
                # (on DVE, off the PE critical path) + gelu.
                ps = pr_ps.tile([128, D], fp32, tag="pr")
                for i, src in enumerate(srcs):
                    ft = pre[i] if pre is not None else load_ft(src, r, nhf)
                    for t in range(nhf):
                        nc.tensor.matmul(ps[:], ft[:, t * 128:(t + 1) * 128],
                                         w_sbs[i][:, t, :],
                                         start=(i == 0 and t == 0),
                                         stop=(i == 1 and t == nhf - 1))
                nc.vector.tensor_add(ps[:], ps[:], b_bc[:])
                # gelu (exact, erf-based) -> stage (bf16)
                nc.scalar.activation(stage[:, r, :], ps[:], AF.Gelu)
                # row stats
                st = ln_p.tile([128, 6], fp32, tag="st")
                nc.vector.bn_stats(out=st[:], in_=stage[:, r, :])
                nc.vector.bn_aggr(out=mv[:, r, :], in_=st[:])

            def ln_finish(nblk, gT, bT, dstT):
                sig = ln_p.tile([128, nblk], fp32, tag="sig")
                nc.scalar.activation(sig[:], mv[:, 0:nblk, 1], AF.Sqrt,
                                     bias=eps_sb[:])
                rstd = ln_p.tile([128, nblk], fp32, tag="rstd")
                nc.vector.reciprocal(rstd[:], sig[:])
                # negmur = -mean * rstd, so ACT can apply the whole
                # normalize step as Identity(x*rstd + negmur)
                negmur = ln_p.tile([128, nblk], fp32, tag="mur")
                nc.vector.tensor_scalar(out=negmur[:], in0=mv[:, 0:nblk, 0],
                                        scalar1=-1.0, scalar2=None, op0=ALU.mult)
                nc.vector.tensor_mul(negmur[:], negmur[:], rstd[:])
                for r in range(nblk):
                    qn = ln_p.tile([128, D], bf16, tag="qn")
                    nc.scalar.activation(qn[:], stage[:, r, :], AF.Identity,
                                         bias=negmur[:, r:r + 1],
                                         scale=rstd[:, r:r + 1])
                    # full-bank tile (2KB/partition): avoids PSUM zero-region
                    # sharing between pool slots
                    ltp = lt_ps.tile([128, 8, 128], bf16, tag="ltp")
                    for dt in range(ND):
                        nc.tensor.transpose(
                            ltp[:, dt, :], qn[:, dt * 128:(dt + 1) * 128], ident[:])
                    # gamma/beta are per-PARTITION in the transposed layout:
                    # fuse them into the PSUM drain as scale+bias
                    for dt in range(ND):
                        dst = dstT[:, dt, r * 128:(r + 1) * 128]
                        if dt % 2 == 0:
                            nc.scalar.activation(dst, ltp[:, dt, :], AF.Identity,
                                                 bias=bT[:, dt:dt + 1],
                                                 scale=gT[:, dt:dt + 1])
                        else:
                            nc.vector.tensor_scalar(
                                out=dst, in0=ltp[:, dt, :],
                                scalar1=gT[:, dt:dt + 1],
                                scalar2=bT[:, dt:dt + 1],
                                op0=ALU.mult, op1=ALU.add)

            # Q first, then K (stage/mv buffers are reused).
            # DMA issue order: block-0 activations, Wq, then Wk chunks
            # streamed behind the first Q blocks' loads.
            NQH = SH // 128   # 8 feature tiles per q source
            NKH = TH // 128   # 32 per k source
            # DMA order: first ft chunk, then the matching weight chunk, so
            # block-0 matmuls start as early as possible
            ft0 = nat_p.tile([128, NQH * 128], bf16, tag="nat")
            nc.gpsimd.dma_start(out=ft0[:, 0:2 * 128],
                                in_=so_d[0, :, 0:2, :].rearrange("p t r -> p (t r)"))
            load_wq_chunk(0)
            nc.gpsimd.dma_start(out=ft0[:, 2 * 128:],
                                in_=so_d[0, :, 2:, :].rearrange("p t r -> p (t r)"))
            ft0b = load_ft(sd_d, 0, NQH, nchunks=2)
            load_wq_chunk(1)
            pre0 = [ft0, ft0b]
            for r in range(NQB):
                proj_block(r, NQH, (so_d, sd_d), (wqo_sb, wqd_sb), bq_bc,
                           pre=pre0 if r == 0 else None)
                if r < 8:
                    load_wk_chunk(r)
            ln_finish(NQB, gqT, betaqT, qT)
            for r in range(NKB):
                proj_block(r, NKH, (to_d, td_d), (wko_sb, wkd_sb), bk_bc)
            ln_finish(NKB, gkT, betakT, kT)

        # ================= phase 2: scores, softmax, z ====================
        with tc.tile_pool(name="sm", bufs=3) as sm_p, \
             tc.tile_pool(name="scps", bufs=2, space="PSUM") as sc_ps, \
             tc.tile_pool(name="atps", bufs=2, space="PSUM") as at_ps, \
             tc.tile_pool(name="zps", bufs=2, space="PSUM") as z_ps:
            for grp in range(NQB):          # 4 batches per group
                ps_s_full = sc_ps.tile([128, 512], fp32, tag="sc")
                ps_s = ps_s_full[:, 0:M]
                for j in range(4):
                    bat = grp * 4 + j
                    for dt in range(ND):
                        nc.tensor.matmul(
                            ps_s[32 * j:32 * (j + 1), :],
                            qT[:, dt, bat * L:(bat + 1) * L],
                            kT[:, dt, bat * M:(bat + 1) * M],
                            start=(dt == 0), stop=(dt == ND - 1),
                            tile_position=(0, 32 * j))
                s_sb = sm_p.tile([128, M], fp32, tag="s")
                nc.vector.tensor_add(s_sb[:], ps_s[:], dbias_sb[:])
                nmax = sm_p.tile([128, 1], fp32, tag="nmax")
                nc.vector.tensor_reduce(out=nmax[:], in_=s_sb[:], axis=AX.X,
                                        op=ALU.max, negate=True)
                e_sb = sm_p.tile([128, M], fp32, tag="e")
                sume = sm_p.tile([128, 1], fp32, tag="sume")
                nc.scalar.activation(e_sb[:], s_sb[:], AF.Exp, bias=nmax[:],
                                     accum_out=sume[:])
                rinv = sm_p.tile([128, 1], fp32, tag="rinv")
                nc.vector.reciprocal(rinv[:], sume[:])
                al_sb = sm_p.tile([128, M], fp32, tag="al")
                nc.vector.tensor_scalar_mul(al_sb[:], e_sb[:], rinv[:])
                al_bf = sm_p.tile([128, M], bf16, tag="albf")
                nc.vector.tensor_scalar_mul(al_bf[:], e_sb[:], rinv[:])
                nc.sync.dma_start(out=al_f[grp * 128:(grp + 1) * 128, :], in_=al_sb[:])
                # alpha^T [48, 128] then z = alpha @ basis
                at_p_full = at_ps.tile([M, 1024], bf16, tag="at")
                at_p = at_p_full[:, 0:128]
                nc.tensor.transpose(at_p[:], al_bf[:], ident[:])
                at_sb = sm_p.tile([M, 128], bf16, tag="atsb")
                drain(at_sb[:], at_p[:])
                ps_z = z_ps.tile([128, D], fp32, tag="z")
                nc.tensor.matmul(ps_z[:], at_sb[:], basis_sb[:], start=True, stop=True)
                z_sb = sm_p.tile([128, D], fp32, tag="zsb")
                drain(z_sb[:], ps_z[:])
                nc.sync.dma_start(out=z_f[grp * 128:(grp + 1) * 128, :], in_=z_sb[:])

    nc.compile()
    return nc


def _get_nc():
    global _NC
    if _NC is None:
        _NC = _build_nc()
    return _NC


def make_in_maps(inputs):
    scale = 1.0 / np.sqrt(D)
    gq_s = (np.asarray(inputs["gq"], np.float32) * scale).astype(np.float32)
    betaq_s = (np.asarray(inputs["betaq"], np.float32) * scale).astype(np.float32)
    l_idx = np.linspace(0.0, 1.0, L, dtype=np.float32)
    t_idx = np.linspace(0.0, 1.0, M, dtype=np.float32)
    dbias = (-LAMBDA_POS * np.abs(l_idx[:, None] - t_idx[None, :])).astype(np.float32)
    dbias128 = np.tile(dbias, (4, 1))  # [128, 48]

    import ml_dtypes
    bf16 = ml_dtypes.bfloat16

    def c(a):
        return np.ascontiguousarray(np.asarray(a, np.float32))

    def swz(a, nblk, nft):
        # [rows, F] -> [block, f, ftile, row] in bf16: one contiguous DMA per
        # block lands features on SBUF partitions (the matmul contraction
        # axis); the host-side bf16 cast halves HBM traffic.
        x = np.asarray(a, np.float32).astype(bf16).reshape(nblk, 128, nft, 128)
        return np.ascontiguousarray(x.transpose(0, 3, 2, 1))

    wq16 = np.ascontiguousarray(np.asarray(inputs["Wq"], np.float32).astype(bf16))
    wk16 = np.ascontiguousarray(np.asarray(inputs["Wk"], np.float32).astype(bf16))
    basis16 = np.ascontiguousarray(
        np.asarray(inputs["basis"], np.float32).astype(bf16))

    in_maps = []
    for i in range(NCORES):
        sl = slice(i * BPC, (i + 1) * BPC)
        in_maps.append({
            "student_oT": swz(inputs["student_o"][sl], NQB, SH // 128),
            "student_dT": swz(inputs["student_d"][sl], NQB, SH // 128),
            "teacher_oT": swz(inputs["teacher_o"][sl], NKB, TH // 128),
            "teacher_dT": swz(inputs["teacher_d"][sl], NKB, TH // 128),
            "Wq": wq16, "Wk": wk16,
            "bq": c(inputs["bq"]), "bk": c(inputs["bk"]),
            "gq_s": gq_s, "betaq_s": betaq_s,
            "gk": c(inputs["gk"]), "betak": c(inputs["betak"]),
            "basis": basis16,
            "depth_bias": dbias128,
        })
    return in_maps


def kernel(**inputs):
    from concourse import bass_utils
    nc = _get_nc()
    in_maps = make_in_maps(inputs)
    res = bass_utils.run_bass_kernel_spmd(nc, in_maps, core_ids=list(range(NCORES)))
    z = np.concatenate([r["z"] for r in res.results], axis=0)
    alpha = np.concatenate([r["alpha"] for r in res.results], axis=0)
    return z.astype(np.float32), alpha.astype(np.float32)


# revision 50
# speedup vs baseline: 1.0520x; 1.0520x over previous
"""Trainium2 Bass kernel for nn_AttentionModule (dense_transformer).

Computes, per batch b:
  q = LN(gelu([student_o;student_d] @ Wq + bq)) * gq + betaq      [B, L, D]
  k = LN(gelu([teacher_o;teacher_d] @ Wk + bk)) * gk + betak      [B, M, D]
  scores = q @ k^T / sqrt(D) + depth_bias                          [B, L, M]
  alpha  = softmax(scores, axis=-1)
  z      = alpha @ basis                                           [B, L, D]
Returns (z, alpha).

Strategy: data-parallel over batch across 8 NeuronCores (64 batches/core).
All matmuls in bf16 (fp32 PSUM accumulation); activations are cast
fp32->bf16 during the SWDGE DMA load and transposed on the PE
(features-on-partitions) since the TensorE contracts along partitions.
1/sqrt(D) is folded into gq/betaq host-side; the projection bias is folded
into the matmul as an extra K=1 rank-1 update (ones x bias).
"""

import sys
import numpy as np

for _p in ("/opt/trn_rl_repo", "/opt/pypackages"):
    if _p not in sys.path:
        sys.path.append(_p)

# ---- problem constants (hardcoded per contract) ----
B, L, M = 512, 32, 48
SH, TH, D = 1024, 4096, 512
LAMBDA_POS = 1.0
EPS = 1e-5
NCORES = 8
BPC = B // NCORES            # 64 batches per core
RQ = BPC * L                 # 2048 q rows per core
RK = BPC * M                 # 3072 k rows per core
FQ = 2 * SH                  # 2048
FK = 2 * TH                  # 8192
NQB = RQ // 128              # 16 q row-blocks
NKB = RK // 128              # 24 k row-blocks
NQF = FQ // 128              # 16 q feature tiles
NKF = FK // 128              # 64 k feature tiles
ND = D // 128                # 4 d tiles

_NC = None


def _build_nc():
    import concourse.tile as tile
    from concourse import bacc, mybir, masks

    fp32 = mybir.dt.float32
    bf16 = mybir.dt.bfloat16
    AF = mybir.ActivationFunctionType
    ALU = mybir.AluOpType
    AX = mybir.AxisListType

    nc = bacc.Bacc("TRN2", target_bir_lowering=False, debug=False,
                   enable_asserts=False, num_devices=NCORES)

    # ---- DRAM I/O ----
    # activations arrive pre-swizzled from the host as [block, f, ftile, row]
    # so a single contiguous DMA per block yields feature-on-partitions tiles
    # (no on-chip transpose needed).
    so_d = nc.dram_tensor("student_oT", [NQB, 128, SH // 128, 128], bf16,
                          kind="ExternalInput").ap()
    sd_d = nc.dram_tensor("student_dT", [NQB, 128, SH // 128, 128], bf16,
                          kind="ExternalInput").ap()
    to_d = nc.dram_tensor("teacher_oT", [NKB, 128, TH // 128, 128], bf16,
                          kind="ExternalInput").ap()
    td_d = nc.dram_tensor("teacher_dT", [NKB, 128, TH // 128, 128], bf16,
                          kind="ExternalInput").ap()
    wq_d = nc.dram_tensor("Wq", [FQ, D], bf16, kind="ExternalInput").ap()
    wk_d = nc.dram_tensor("Wk", [FK, D], bf16, kind="ExternalInput").ap()
    bq_d = nc.dram_tensor("bq", [D], fp32, kind="ExternalInput").ap()
    bk_d = nc.dram_tensor("bk", [D], fp32, kind="ExternalInput").ap()
    gq_d = nc.dram_tensor("gq_s", [D], fp32, kind="ExternalInput").ap()
    betaq_d = nc.dram_tensor("betaq_s", [D], fp32, kind="ExternalInput").ap()
    gk_d = nc.dram_tensor("gk", [D], fp32, kind="ExternalInput").ap()
    betak_d = nc.dram_tensor("betak", [D], fp32, kind="ExternalInput").ap()
    basis_d = nc.dram_tensor("basis", [M, D], bf16, kind="ExternalInput").ap()
    dbias_d = nc.dram_tensor("depth_bias", [128, M], fp32, kind="ExternalInput").ap()
    z_d = nc.dram_tensor("z", [BPC, L, D], fp32, kind="ExternalOutput").ap()
    al_d = nc.dram_tensor("alpha", [BPC, L, M], fp32, kind="ExternalOutput").ap()

    z_f = z_d.flatten_outer_dims()     # [2048, 512]
    al_f = al_d.flatten_outer_dims()   # [2048, 48]

    import contextlib
    with tile.TileContext(nc) as tc, contextlib.ExitStack() as ctx:
        const = ctx.enter_context(tc.tile_pool(name="const", bufs=1))
        stage_p = ctx.enter_context(tc.tile_pool(name="stage", bufs=1))
        qkt_p = ctx.enter_context(tc.tile_pool(name="qkt", bufs=1))

        # ---- constants ----
        ident = const.tile([128, 128], bf16)
        masks.make_identity(nc, ident[:])
        eps_sb = const.tile([128, 1], fp32)
        nc.gpsimd.memset(eps_sb[:], float(EPS))

        # weight tiles are allocated per SOURCE (finer dependency grains);
        # their DMAs are issued inside phase 1 (Wq right after the first
        # activation loads, Wk chunks streamed during the Q phase) to avoid
        # a PE startup stall.
        wqo_sb = const.tile([128, SH // 128, D], bf16)   # [128, 8, 512]
        wqd_sb = const.tile([128, SH // 128, D], bf16)
        wko_sb = const.tile([128, TH // 128, D], bf16)   # [128, 32, 512]
        wkd_sb = const.tile([128, TH // 128, D], bf16)

        wq_r = wq_d.rearrange("(t p) d -> p t d", p=128)   # [128, 16, 512]
        wk_r = wk_d.rearrange("(t p) d -> p t d", p=128)   # [128, 64, 512]

        def load_wq_chunk(c):   # c in 0..1 -> one source each
            dst = (wqo_sb, wqd_sb)[c]
            nc.gpsimd.dma_start(out=dst[:], in_=wq_r[:, c * 8:(c + 1) * 8, :])

        def load_wk_chunk(c):   # c in 0..7; 0-3 -> wko, 4-7 -> wkd
            dst = wko_sb if c < 4 else wkd_sb
            cc = c % 4
            nc.gpsimd.dma_start(out=dst[:, cc * 8:(cc + 1) * 8, :],
                                in_=wk_r[:, c * 8:(c + 1) * 8, :])

        import concourse.bass as bass

        def bcast128(src_ap, n):
            return bass.AP(tensor=src_ap.tensor, offset=src_ap.offset,
                           ap=[[0, 128]] + src_ap.ap)

        bq_bc = const.tile([128, D], fp32)
        nc.gpsimd.dma_start(out=bq_bc[:], in_=bcast128(bq_d, D))
        bk_bc = const.tile([128, D], fp32)
        nc.gpsimd.dma_start(out=bk_bc[:], in_=bcast128(bk_d, D))
        gq_bc = const.tile([128, D], fp32)
        nc.gpsimd.dma_start(out=gq_bc[:], in_=bcast128(gq_d, D))
        betaq_bc = const.tile([128, D], fp32)
        nc.gpsimd.dma_start(out=betaq_bc[:], in_=bcast128(betaq_d, D))
        gk_bc = const.tile([128, D], fp32)
        nc.gpsimd.dma_start(out=gk_bc[:], in_=bcast128(gk_d, D))
        betak_bc = const.tile([128, D], fp32)
        nc.gpsimd.dma_start(out=betak_bc[:], in_=bcast128(betak_d, D))
        basis_sb = const.tile([M, D], bf16)
        nc.gpsimd.dma_start(out=basis_sb[:], in_=basis_d)
        dbias_sb = const.tile([128, M], fp32)
        nc.gpsimd.dma_start(out=dbias_sb[:], in_=dbias_d)

        # ---- persistent stage / output tensors (Q and K share one buffer:
        # Q uses rows [0,16), fully consumed before K writes rows [0,24)) ----
        stage = stage_p.tile([128, NKB, D], bf16)      # 24 KB/part
        mv = stage_p.tile([128, NKB, 2], fp32)
        qT = qkt_p.tile([128, ND, RQ], bf16)           # q^T (d on part), 16 KB/part
        kT = qkt_p.tile([128, ND, RK], bf16)           # k^T, 24 KB/part

        drain_eng = [0]

        def drain(out_ap, in_ap):
            # alternate PSUM->SBUF drains between ACT and DVE
            if drain_eng[0] % 2 == 0:
                nc.scalar.activation(out_ap, in_ap, AF.Copy)
            else:
                nc.vector.tensor_copy(out_ap, in_ap)
            drain_eng[0] += 1

        # ================= phase 1: projections + gelu + stats ============
        with tc.tile_pool(name="nat", bufs=5) as nat_p, \
             tc.tile_pool(name="ln", bufs=3) as ln_p, \
             tc.tile_pool(name="prps", bufs=3, space="PSUM") as pr_ps, \
             tc.tile_pool(name="ltps", bufs=2, space="PSUM") as lt_ps:

            def load_ft(src, r, nhf, nchunks=1):
                # contiguous cast-DMA(s): [128 f, nhf*128 rows] bf16 tile,
                # feature already on partitions (host pre-swizzled)
                ft = nat_p.tile([128, nhf * 128], bf16, tag="nat")
                step = nhf // nchunks
                for c in range(nchunks):
                    nc.gpsimd.dma_start(
                        out=ft[:, c * step * 128:(c + 1) * step * 128],
                        in_=src[r, :, c * step:(c + 1) * step, :].rearrange(
                            "p t r -> p (t r)"))
                return ft

            def proj_block(r, nhf, srcs, w_sbs, b_bc, pre=None):
                # per source tensor: accumulate the projection over feature
                # tiles straight out of the DMA-ed featT tile; then bias
                # (on DVE, off the PE critical path) + gelu.
                ps = pr_ps.tile([128, D], fp32, tag="pr")
                for i, src in enumerate(srcs):
                    ft = pre[i] if pre is not None else load_ft(src, r, nhf)
                    for t in range(nhf):
                        nc.tensor.matmul(ps[:], ft[:, t * 128:(t + 1) * 128],
                                         w_sbs[i][:, t, :],
                                         start=(i == 0 and t == 0),
                                         stop=(i == 1 and t == nhf - 1))
                nc.vector.tensor_add(ps[:], ps[:], b_bc[:])
                # gelu (exact, erf-based) -> stage (bf16)
                nc.scalar.activation(stage[:, r, :], ps[:], AF.Gelu)
                # row stats
                st = ln_p.tile([128, 6], fp32, tag="st")
                nc.vector.bn_stats(out=st[:], in_=stage[:, r, :])
                nc.vector.bn_aggr(out=mv[:, r, :], in_=st[:])

            def ln_finish(nblk, g_bc, beta_bc, dstT):
                sig = ln_p.tile([128, nblk], fp32, tag="sig")
                nc.scalar.activation(sig[:], mv[:, 0:nblk, 1], AF.Sqrt,
                                     bias=eps_sb[:])
                rstd = ln_p.tile([128, nblk], fp32, tag="rstd")
                nc.vector.reciprocal(rstd[:], sig[:])
                # negmur = -mean * rstd, so ACT can apply the whole
                # normalize step as Identity(x*rstd + negmur)
                negmur = ln_p.tile([128, nblk], fp32, tag="mur")
                nc.vector.tensor_scalar(out=negmur[:], in0=mv[:, 0:nblk, 0],
                                        scalar1=-1.0, scalar2=None, op0=ALU.mult)
                nc.vector.tensor_mul(negmur[:], negmur[:], rstd[:])
                for r in range(nblk):
                    t0 = ln_p.tile([128, D], fp32, tag="t0")
                    nc.scalar.activation(t0[:], stage[:, r, :], AF.Identity,
                                         bias=negmur[:, r:r + 1],
                                         scale=rstd[:, r:r + 1])
                    nc.vector.tensor_mul(t0[:], t0[:], g_bc[:])
                    qn = ln_p.tile([128, D], bf16, tag="qn")
                    nc.vector.tensor_add(qn[:], t0[:], beta_bc[:])
                    # full-bank tile (2KB/partition): avoids PSUM zero-region
                    # sharing between pool slots
                    ltp = lt_ps.tile([128, 8, 128], bf16, tag="ltp")
                    for dt in range(ND):
                        nc.tensor.transpose(
                            ltp[:, dt, :], qn[:, dt * 128:(dt + 1) * 128], ident[:])
                    drain(dstT[:, :, r * 128:(r + 1) * 128], ltp[:, 0:ND, :])

            # Q first, then K (stage/mv buffers are reused).
            # DMA issue order: block-0 activations, Wq, then Wk chunks
            # streamed behind the first Q blocks' loads.
            NQH = SH // 128   # 8 feature tiles per q source
            NKH = TH // 128   # 32 per k source
            # DMA order: first ft chunk, then the matching weight chunk, so
            # block-0 matmuls start as early as possible
            ft0 = nat_p.tile([128, NQH * 128], bf16, tag="nat")
            nc.gpsimd.dma_start(out=ft0[:, 0:2 * 128],
                                in_=so_d[0, :, 0:2, :].rearrange("p t r -> p (t r)"))
            load_wq_chunk(0)
            nc.gpsimd.dma_start(out=ft0[:, 2 * 128:],
                                in_=so_d[0, :, 2:, :].rearrange("p t r -> p (t r)"))
            ft0b = load_ft(sd_d, 0, NQH, nchunks=2)
            load_wq_chunk(1)
            pre0 = [ft0, ft0b]
            for r in range(NQB):
                proj_block(r, NQH, (so_d, sd_d), (wqo_sb, wqd_sb), bq_bc,
                           pre=pre0 if r == 0 else None)
                if r < 8:
                    load_wk_chunk(r)
            ln_finish(NQB, gq_bc, betaq_bc, qT)
            for r in range(NKB):
                proj_block(r, NKH, (to_d, td_d), (wko_sb, wkd_sb), bk_bc)
            ln_finish(NKB, gk_bc, betak_bc, kT)

        # ================= phase 2: scores, softmax, z ====================
        with tc.tile_pool(name="sm", bufs=3) as sm_p, \
             tc.tile_pool(name="scps", bufs=2, space="PSUM") as sc_ps, \
             tc.tile_pool(name="atps", bufs=2, space="PSUM") as at_ps, \
             tc.tile_pool(name="zps", bufs=2, space="PSUM") as z_ps:
            for grp in range(NQB):          # 4 batches per group
                ps_s_full = sc_ps.tile([128, 512], fp32, tag="sc")
                ps_s = ps_s_full[:, 0:M]
                for j in range(4):
                    bat = grp * 4 + j
                    for dt in range(ND):
                        nc.tensor.matmul(
                            ps_s[32 * j:32 * (j + 1), :],
                            qT[:, dt, bat * L:(bat + 1) * L],
                            kT[:, dt, bat * M:(bat + 1) * M],
                            start=(dt == 0), stop=(dt == ND - 1),
                            tile_position=(0, 32 * j))
                s_sb = sm_p.tile([128, M], fp32, tag="s")
                nc.vector.tensor_add(s_sb[:], ps_s[:], dbias_sb[:])
                nmax = sm_p.tile([128, 1], fp32, tag="nmax")
                nc.vector.tensor_reduce(out=nmax[:], in_=s_sb[:], axis=AX.X,
                                        op=ALU.max, negate=True)
                e_sb = sm_p.tile([128, M], fp32, tag="e")
                sume = sm_p.tile([128, 1], fp32, tag="sume")
                nc.scalar.activation(e_sb[:], s_sb[:], AF.Exp, bias=nmax[:],
                                     accum_out=sume[:])
                rinv = sm_p.tile([128, 1], fp32, tag="rinv")
                nc.vector.reciprocal(rinv[:], sume[:])
                al_sb = sm_p.tile([128, M], fp32, tag="al")
                nc.vector.tensor_scalar_mul(al_sb[:], e_sb[:], rinv[:])
                al_bf = sm_p.tile([128, M], bf16, tag="albf")
                nc.vector.tensor_scalar_mul(al_bf[:], e_sb[:], rinv[:])
                nc.sync.dma_start(out=al_f[grp * 128:(grp + 1) * 128, :], in_=al_sb[:])
                # alpha^T [48, 128] then z = alpha @ basis
                at_p_full = at_ps.tile([M, 1024], bf16, tag="at")
                at_p = at_p_full[:, 0:128]
                nc.tensor.transpose(at_p[:], al_bf[:], ident[:])
                at_sb = sm_p.tile([M, 128], bf16, tag="atsb")
                drain(at_sb[:], at_p[:])
                ps_z = z_ps.tile([128, D], fp32, tag="z")
                nc.tensor.matmul(ps_z[:], at_sb[:], basis_sb[:], start=True, stop=True)
                z_sb = sm_p.tile([128, D], fp32, tag="zsb")
                drain(z_sb[:], ps_z[:])
                nc.sync.dma_start(out=z_f[grp * 128:(grp + 1) * 128, :], in_=z_sb[:])

    nc.compile()
    return nc


def _get_nc():
    global _NC
    if _NC is None:
        _NC = _build_nc()
    return _NC


def make_in_maps(inputs):
    scale = 1.0 / np.sqrt(D)
    gq_s = (np.asarray(inputs["gq"], np.float32) * scale).astype(np.float32)
    betaq_s = (np.asarray(inputs["betaq"], np.float32) * scale).astype(np.float32)
    l_idx = np.linspace(0.0, 1.0, L, dtype=np.float32)
    t_idx = np.linspace(0.0, 1.0, M, dtype=np.float32)
    dbias = (-LAMBDA_POS * np.abs(l_idx[:, None] - t_idx[None, :])).astype(np.float32)
    dbias128 = np.tile(dbias, (4, 1))  # [128, 48]

    import ml_dtypes
    bf16 = ml_dtypes.bfloat16

    def c(a):
        return np.ascontiguousarray(np.asarray(a, np.float32))

    def swz(a, nblk, nft):
        # [rows, F] -> [block, f, ftile, row] in bf16: one contiguous DMA per
        # block lands features on SBUF partitions (the matmul contraction
        # axis); the host-side bf16 cast halves HBM traffic.
        x = np.asarray(a, np.float32).astype(bf16).reshape(nblk, 128, nft, 128)
        return np.ascontiguousarray(x.transpose(0, 3, 2, 1))

    wq16 = np.ascontiguousarray(np.asarray(inputs["Wq"], np.float32).astype(bf16))
    wk16 = np.ascontiguousarray(np.asarray(inputs["Wk"], np.float32).astype(bf16))
    basis16 = np.ascontiguousarray(
        np.asarray(inputs["basis"], np.float32).astype(bf16))

    in_maps = []
    for i in range(NCORES):
        sl = slice(i * BPC, (i + 1) * BPC)
        in_maps.append({
            "student_oT": swz(inputs["student_o"][sl], NQB, SH // 128),
            "student_dT": swz(inputs["student_d"][sl], NQB, SH // 128),
            "teacher_oT": swz(inputs["teacher_o"][sl], NKB, TH // 128),
            "teacher_dT": swz(inputs["teacher_d"][sl], NKB, TH // 128),
            "Wq": wq16, "Wk": wk16,
            "bq": c(inputs["bq"]), "bk": c(inputs["bk"]),
            "gq_s": gq_s, "betaq_s": betaq_s,
            "gk": c(inputs["gk"]), "betak": c(inputs["betak"]),
            "basis": basis16,
            "depth_bias": dbias128,
        })
    return in_maps


def kernel(**inputs):
    from concourse import bass_utils
    nc = _get_nc()
    in_maps = make_in_maps(inputs)
    res = bass_utils.run_bass_kernel_spmd(nc, in_maps, core_ids=list(range(NCORES)))
    z = np.concatenate([r["z"] for r in res.results], axis=0)
    alpha = np.concatenate([r["alpha"] for r in res.results], axis=0)
    return z.astype(np.float32), alpha.astype(np.float32)
